# revision 37
# baseline (speedup 1.0000x reference)
"""Expert-parallel MoE MLP (Llama4 text experts) for 8 Trainium2 NeuronCores.

Strategy: core e handles expert e. Tokens are grouped by expert on the host
(indices are sorted; argsort for robustness), padded to T_pad = max expert
count (rounded to 8), and each core runs a dense gated MLP over its block:
    y = (up * silu(gate)) @ W_d,   [gate | up] = x @ W_gu
computed transposed (weights stationary, tokens streaming):
    y^T = W_d^T @ h^T,  h^T = up^T * silu(W_gu^T @ x^T)
bf16 inputs, fp32 PSUM accumulation, fp32 output.

Perf notes vs the naive version:
  - All inputs land via ~10 large contiguous DMAs (0.5-4 MB, 4-32 KB
    per-partition lines) instead of 128 small strided ones.
  - The SDMA engines round-robin every in-flight DMA at packet granularity, so
    only the critical set (x block0 + pair-0 weights, 1.5 MB) is issued
    immediately; the rest is released in a staggered chain (GpSimd copy links)
    in consumption order, a few DMAs in flight at a time.
  - Gate/up weight columns are pair-interleaved on the host so weights are
    consumed in DMA arrival order.
  - T_pad is the max expert count rounded to 4 (not 128), split into blocks
    (512, r, r) so there is no tiny tail block.
  - 44 dummy matmuls on zeroed SBUF run during the initial DMA wait to lift
    the PE HAM clock gate (cold 1.2 GHz -> warm 2.4 GHz) before the real
    matmul stream begins; the stream then runs at the bf16 PE roofline
    (~216 ns per 128x128x512 matmul) with no re-throttle.
"""

import numpy as np
import ml_dtypes

_BF16 = ml_dtypes.bfloat16
_NC = 8  # cores

_nc_cache: dict = {}
last_run = None  # BassKernelResults of the most recent kernel() call


def _build(T_pad: int, H: int, F: int):
    import concourse.bacc as bacc
    import concourse.mybir as mybir
    from concourse.tile import TileContext

    nc = bacc.Bacc()
    KB1 = H // 128        # contraction chunks for x @ W_gu (8)
    NPAIR = F // 128      # gate/up pairs (16)
    KB2 = F // 128        # contraction chunks for h @ W_d (16)
    NH = H // 128         # output tiles of y (8)
    B0 = 512              # block0 tokens
    T_r = T_pad - B0      # tokens in blocks 1+2
    nb1 = T_r // 2
    blocks = [(0, B0), (B0, nb1), (B0 + nb1, T_r - nb1)]

    bf16 = mybir.dt.bfloat16
    f32 = mybir.dt.float32

    # DRAM inputs: host-packed so every DMA is a whole-tensor contiguous copy.
    KH = KB1 // 2
    xb0a = nc.dram_tensor("xb0a", [128, KH * B0], bf16, kind="ExternalInput")
    xb0b = nc.dram_tensor("xb0b", [128, KH * B0], bf16, kind="ExternalInput")
    xb12 = nc.dram_tensor("xb12", [128, KB1 * T_r], bf16, kind="ExternalInput")
    wp0g = nc.dram_tensor("wp0g", [128, KB1 * 128], bf16, kind="ExternalInput")
    wp0u = nc.dram_tensor("wp0u", [128, KB1 * 128], bf16, kind="ExternalInput")
    wp1 = nc.dram_tensor("wp1", [128, KB1 * 256], bf16, kind="ExternalInput")
    whd2 = nc.dram_tensor("whd2", [128, KB1 * 512], bf16, kind="ExternalInput")
    wgg = [
        nc.dram_tensor(f"wgg{g}", [128, KB1 * 1024], bf16, kind="ExternalInput")
        for g in range(3)
    ]
    wdp = nc.dram_tensor("wdp", [128, KB2 * 1024], bf16, kind="ExternalInput")
    yT = nc.dram_tensor("yT", [128, NH * T_pad], f32, kind="ExternalOutput")

    def gu_lhsT(p, k, up):
        # lhsT [128,128] for pair p (gate if not up), contraction chunk k
        off = 128 if up else 0
        if p == 0:
            base = KB1 * 128 if up else 0
            return wp_sb[0][:, base + k * 128 : base + k * 128 + 128]
        if p == 1:
            return wp_sb[1][:, k * 256 + off : k * 256 + off + 128]
        if p < 4:
            q = p - 2
            return whd2_sb[:, k * 512 + 256 * q + off : k * 512 + 256 * q + off + 128]
        g, q = (p - 4) // 4, (p - 4) % 4
        return wg_sb[g][:, k * 1024 + 256 * q + off : k * 1024 + 256 * q + off + 128]

    def x_rhs(b, k):
        t0, nb = blocks[b]
        if b == 0:
            return x0_sb[:, k * B0 : k * B0 + nb]
        o = t0 - B0
        return x12_sb[:, k * T_r + o : k * T_r + o + nb]

    with TileContext(nc) as tc:
        with (
            tc.tile_pool(name="win", bufs=1) as win_p,
            tc.tile_pool(name="warm", bufs=1) as warm_p,
            tc.tile_pool(name="silu_p", bufs=3) as silu_p,
            tc.tile_pool(name="h_p", bufs=2) as h_p,
            tc.tile_pool(name="y_p", bufs=3) as y_p,
            tc.tile_pool(name="ps1", bufs=5, space="PSUM") as ps1_p,
            tc.tile_pool(name="ps2", bufs=3, space="PSUM") as ps2_p,
        ):
            # PE warm-up: dummy matmuls on zeroed SBUF while inputs stream
            # in, lifting the HAM clock gate (cold 1.2 GHz -> warm 2.4 GHz)
            # before the real matmul stream. Results land in the ps2 rotation
            # and are overwritten by later start=True accumulation groups.
            # Warm matmuls 24 and 36 double as timers for the release chain.
            wtile = warm_p.tile([128, 256], bf16, name="wtile", tag="wtile")
            nc.vector.memset(wtile[:], 0.0)
            ps_g24 = ps_g36 = None
            for i in range(44):
                ps_w = ps2_p.tile([128, 512], f32, tag="ps2")
                nc.tensor.matmul(
                    out=ps_w[:, :128],
                    lhsT=wtile[:, 128:256],
                    rhs=wtile[:, 0:128],
                    start=True,
                    stop=True,
                )
                if i == 24:
                    ps_g24 = ps_w
                if i == 36:
                    ps_g36 = ps_w

            # Critical input DMAs (x block0 + pair-0 weights, 1.5 MB): issued
            # via SWDGE on the GpSimd queue, which clears the framework
            # preamble ~1.5us before the Sync/Scalar HWDGE rings — first bytes
            # hit the wire earlier. Ordered by need: wp0, then x0 halves.
            x0_sb = win_p.tile([128, KB1 * B0], bf16, name="x0", tag="x0")
            wp_sb = []
            for p in range(2):
                t = win_p.tile([128, KB1 * 256], bf16, name=f"wp{p}", tag=f"wp{p}")
                wp_sb.append(t)
            nc.gpsimd.dma_start(out=wp_sb[0][:, : KB1 * 128], in_=wp0g[:, :])
            nc.gpsimd.dma_start(out=x0_sb[:, : KH * B0], in_=xb0a[:, :])
            nc.gpsimd.dma_start(out=x0_sb[:, KH * B0 :], in_=xb0b[:, :])
            nc.gpsimd.dma_start(out=wp_sb[0][:, KB1 * 128 :], in_=wp0u[:, :])
            gate_t = win_p.tile([128, 8], f32, name="gate_t", tag="gate_t")
            nc.vector.tensor_copy(gate_t[:, :4], ps_g24[:, :4])
            nc.vector.tensor_copy(gate_t[:, 4:8], ps_g36[:, :4])

            # Staggered release of the bulk DMAs, a few in flight at a time,
            # in consumption order. Each link: a GpSimd copy into the head of
            # the next DMA's dest tile, gated on data a previous DMA wrote;
            # the dma_start then has a write-after-write dependency on that
            # copy. This keeps the SDMA packet round-robin from letting bulk
            # transfers starve urgent ones.
            whd2_sb = win_p.tile([128, KB1 * 512], bf16, name="whd2", tag="whd2")
            x12_sb = win_p.tile([128, KB1 * T_r], bf16, name="x12", tag="x12")
            wg_sb = [
                win_p.tile([128, KB1 * 1024], bf16, name=f"wg{g}", tag=f"wg{g}")
                for g in range(3)
            ]
            wd_sb = win_p.tile([128, KB2 * 1024], bf16, name="wd", tag="wd")

            def release(dst_tile, src_ap):
                nc.gpsimd.tensor_copy(dst_tile[:, :4], src_ap)

            release(wp_sb[1], gate_t[:, :4])        # after warm matmul 24
            nc.sync.dma_start(out=wp_sb[1], in_=wp1[:, :])
            release(whd2_sb, gate_t[:, 4:8])        # after warm matmul 36
            nc.sync.dma_start(out=whd2_sb, in_=whd2[:, :])
            release(wg_sb[0], gate_t[:, 4:8])       # after warm matmul 36
            nc.sync.dma_start(out=wg_sb[0], in_=wgg[0][:, :])
            release(wg_sb[1], wp_sb[1][:, 8:12])    # after wp1 lands
            nc.sync.dma_start(out=wg_sb[1], in_=wgg[1][:, :])
            release(wg_sb[2], whd2_sb[:, 8:12])     # after whd2 lands
            nc.sync.dma_start(out=wg_sb[2], in_=wgg[2][:, :])
            release(wd_sb, wg_sb[0][:, 8:12])       # after wg0 lands
            nc.sync.dma_start(out=wd_sb, in_=wdp[:, :])
            release(x12_sb, wg_sb[0][:, 12:16])     # after wg0 lands
            nc.scalar.dma_start(out=x12_sb, in_=xb12[:, :])

            h_tiles = {}

            def gateup(b):
                t0, nb = blocks[b]
                for p in range(NPAIR):
                    ps_g = ps1_p.tile([128, 512], f32, tag="ps1")
                    for k in range(KB1):
                        nc.tensor.matmul(
                            out=ps_g[:, :nb],
                            lhsT=gu_lhsT(p, k, False),
                            rhs=x_rhs(b, k),
                            start=(k == 0),
                            stop=(k == KB1 - 1),
                        )
                    ps_u = ps1_p.tile([128, 512], f32, tag="ps1")
                    for k in range(KB1):
                        nc.tensor.matmul(
                            out=ps_u[:, :nb],
                            lhsT=gu_lhsT(p, k, True),
                            rhs=x_rhs(b, k),
                            start=(k == 0),
                            stop=(k == KB1 - 1),
                        )
                    st = silu_p.tile([128, 512], bf16, tag="silu")
                    nc.scalar.activation(
                        st[:, :nb], ps_g[:, :nb], mybir.ActivationFunctionType.Silu
                    )
                    ht = h_p.tile([128, 512], bf16, tag=f"h{p}")
                    nc.vector.tensor_mul(out=ht[:, :nb], in0=ps_u[:, :nb], in1=st[:, :nb])
                    h_tiles[(b, p)] = ht

            def down(b, split_last=False):
                t0, nb = blocks[b]
                for hh in range(NH):
                    # Split the final output tile into two column halves so
                    # the first half's copy+DMA overlaps the second half's
                    # matmuls, shortening the kernel tail.
                    halves = [(0, nb)]
                    if split_last and hh == NH - 1:
                        halves = [(0, nb // 2), (nb // 2, nb - nb // 2)]
                    for ci, (c0, cn) in enumerate(halves):
                        ps_y = ps2_p.tile([128, 512], f32, tag="ps2")
                        for k in range(KB2):
                            nc.tensor.matmul(
                                out=ps_y[:, :cn],
                                lhsT=wd_sb[:, k * 1024 + 128 * hh : k * 1024 + 128 * hh + 128],
                                rhs=h_tiles[(b, k)][:, c0 : c0 + cn],
                                start=(k == 0),
                                stop=(k == KB2 - 1),
                            )
                        yt = y_p.tile([128, 512], f32, tag="y")
                        nc.vector.tensor_copy(yt[:, :cn], ps_y[:, :cn])
                        nc.sync.dma_start(
                            out=yT[:, hh * T_pad + t0 + c0 : hh * T_pad + t0 + c0 + cn],
                            in_=yt[:, :cn],
                        )

            gateup(0)
            gateup(1)
            down(0)
            down(1)
            gateup(2)
            down(2, split_last=True)
    nc.compile()
    return nc


def kernel(hidden_states, local_expert_indices, gate_up_proj, down_proj):
    from concourse.bass_utils import run_bass_kernel_spmd

    x = np.asarray(hidden_states, dtype=np.float32)
    idx = np.asarray(local_expert_indices).astype(np.int64)
    wgu_all = np.asarray(gate_up_proj, dtype=np.float32)
    wd_all = np.asarray(down_proj, dtype=np.float32)

    T, H = x.shape
    E, _, F2 = wgu_all.shape
    F = F2 // 2
    assert E == _NC
    KB1 = H // 128

    order = np.argsort(idx, kind="stable")
    counts = np.bincount(idx, minlength=E)
    starts = np.concatenate([[0], np.cumsum(counts)])
    B0 = 512
    T_pad = max(B0 + 8, int(-(-counts.max() // 4) * 4))
    if (T_pad - B0) % 2:
        T_pad += 2
    T_r = T_pad - B0

    key = (T_pad, H, F)
    if key not in _nc_cache:
        _nc_cache[key] = _build(T_pad, H, F)
    nc = _nc_cache[key]

    x_sorted = x[order]
    in_maps = []
    for e in range(E):
        s, c = int(starts[e]), int(counts[e])
        xb = np.zeros((T_pad, H), np.float32)
        xb[:c] = x_sorted[s : s + c]
        xb = xb.astype(_BF16)
        # x chunk-major: [128 p][KB1 k][t]
        xb0 = np.ascontiguousarray(
            xb[:B0].reshape(B0, KB1, 128).transpose(2, 1, 0)
        ).reshape(128, KB1 * B0)
        xb0a = np.ascontiguousarray(xb0[:, : (KB1 // 2) * B0])
        xb0b = np.ascontiguousarray(xb0[:, (KB1 // 2) * B0 :])
        xb12 = np.ascontiguousarray(
            xb[B0:].reshape(T_r, KB1, 128).transpose(2, 1, 0)
        ).reshape(128, KB1 * T_r)
        # gate/up pair-interleaved columns: packed col block 256p = [gate_p | up_p]
        w = wgu_all[e].astype(_BF16)
        wg_ = w[:, :F].reshape(H, F // 128, 128)
        wu_ = w[:, F:].reshape(H, F // 128, 128)
        wp = np.empty((H, F // 128, 2, 128), _BF16)
        wp[:, :, 0] = wg_
        wp[:, :, 1] = wu_
        wp = wp.reshape(H, 2 * F)
        wpr = wp.reshape(KB1, 128, 2 * F).transpose(1, 0, 2)  # [p][k][c]
        wp0g = np.ascontiguousarray(wpr[:, :, 0:128]).reshape(128, KB1 * 128)
        wp0u = np.ascontiguousarray(wpr[:, :, 128:256]).reshape(128, KB1 * 128)
        wp1 = np.ascontiguousarray(wpr[:, :, 256:512]).reshape(128, KB1 * 256)
        whd2 = np.ascontiguousarray(wpr[:, :, 512:1024]).reshape(128, KB1 * 512)
        wggs = {
            f"wgg{g}": np.ascontiguousarray(
                wpr[:, :, 1024 + 1024 * g : 2048 + 1024 * g]
            ).reshape(128, KB1 * 1024)
            for g in range(3)
        }
        wdp = np.ascontiguousarray(
            wd_all[e].astype(_BF16).reshape(F // 128, 128, H).transpose(1, 0, 2)
        ).reshape(128, (F // 128) * H)
        in_maps.append(
            {"xb0a": xb0a, "xb0b": xb0b, "xb12": xb12, "wp0g": wp0g, "wp0u": wp0u, "wp1": wp1,
             "whd2": whd2, "wdp": wdp, **wggs}
        )

    res = run_bass_kernel_spmd(nc, in_maps, core_ids=list(range(_NC)))
    global last_run
    last_run = res

    out = np.zeros((T, H), np.float32)
    for e in range(E):
        s, c = int(starts[e]), int(counts[e])
        if c:
            ye = np.asarray(res.results[e]["yT"]).reshape(128, H // 128, T_pad)
            y_pad = ye.transpose(2, 1, 0).reshape(T_pad, H)
            out[order[s : s + c]] = y_pad[:c]
    return out


# revision 38
# speedup vs baseline: 1.0032x; 1.0032x over previous
"""Expert-parallel MoE MLP (Llama4 text experts) for 8 Trainium2 NeuronCores.

Strategy: core e handles expert e. Tokens are grouped by expert on the host
(indices are sorted; argsort for robustness), padded to T_pad = max expert
count (rounded to 8), and each core runs a dense gated MLP over its block:
    y = (up * silu(gate)) @ W_d,   [gate | up] = x @ W_gu
computed transposed (weights stationary, tokens streaming):
    y^T = W_d^T @ h^T,  h^T = up^T * silu(W_gu^T @ x^T)
bf16 inputs, fp32 PSUM accumulation, fp32 output.

Perf notes vs the naive version:
  - All inputs land via ~10 large contiguous DMAs (0.5-4 MB, 4-32 KB
    per-partition lines) instead of 128 small strided ones.
  - The SDMA engines round-robin every in-flight DMA at packet granularity, so
    only the critical set (x block0 + pair-0 weights, 1.5 MB) is issued
    immediately; the rest is released in a staggered chain (GpSimd copy links)
    in consumption order, a few DMAs in flight at a time.
  - Gate/up weight columns are pair-interleaved on the host so weights are
    consumed in DMA arrival order.
  - T_pad is the max expert count rounded to 4 (not 128), split into blocks
    (512, r, r) so there is no tiny tail block.
  - 44 dummy matmuls on zeroed SBUF run during the initial DMA wait to lift
    the PE HAM clock gate (cold 1.2 GHz -> warm 2.4 GHz) before the real
    matmul stream begins; the stream then runs at the bf16 PE roofline
    (~216 ns per 128x128x512 matmul) with no re-throttle.
"""

import numpy as np
import ml_dtypes

_BF16 = ml_dtypes.bfloat16
_NC = 8  # cores

_nc_cache: dict = {}
last_run = None  # BassKernelResults of the most recent kernel() call


def _build(T_pad: int, H: int, F: int):
    import concourse.bacc as bacc
    import concourse.mybir as mybir
    from concourse.tile import TileContext

    nc = bacc.Bacc()
    KB1 = H // 128        # contraction chunks for x @ W_gu (8)
    NPAIR = F // 128      # gate/up pairs (16)
    KB2 = F // 128        # contraction chunks for h @ W_d (16)
    NH = H // 128         # output tiles of y (8)
    B0 = 512              # block0 tokens
    T_r = T_pad - B0      # tokens in blocks 1+2
    nb1 = T_r // 2
    blocks = [(0, B0), (B0, nb1), (B0 + nb1, T_r - nb1)]

    bf16 = mybir.dt.bfloat16
    f32 = mybir.dt.float32

    # DRAM inputs: host-packed so every DMA is a whole-tensor contiguous copy.
    KH = KB1 // 2
    xb0a = nc.dram_tensor("xb0a", [128, KH * B0], bf16, kind="ExternalInput")
    xb0b = nc.dram_tensor("xb0b", [128, KH * B0], bf16, kind="ExternalInput")
    xb12 = nc.dram_tensor("xb12", [128, KB1 * T_r], bf16, kind="ExternalInput")
    wp0 = nc.dram_tensor("wp0", [128, KB1 * 256], bf16, kind="ExternalInput")
    wp1 = nc.dram_tensor("wp1", [128, KB1 * 256], bf16, kind="ExternalInput")
    whd2 = nc.dram_tensor("whd2", [128, KB1 * 512], bf16, kind="ExternalInput")
    wgg = [
        nc.dram_tensor(f"wgg{g}", [128, KB1 * 1024], bf16, kind="ExternalInput")
        for g in range(3)
    ]
    wdp = nc.dram_tensor("wdp", [128, KB2 * 1024], bf16, kind="ExternalInput")
    yT = nc.dram_tensor("yT", [128, NH * T_pad], f32, kind="ExternalOutput")

    def gu_lhsT(p, k, up):
        # lhsT [128,128] for pair p (gate if not up), contraction chunk k
        off = 128 if up else 0
        if p < 2:
            return wp_sb[p][:, k * 256 + off : k * 256 + off + 128]
        if p < 4:
            q = p - 2
            return whd2_sb[:, k * 512 + 256 * q + off : k * 512 + 256 * q + off + 128]
        g, q = (p - 4) // 4, (p - 4) % 4
        return wg_sb[g][:, k * 1024 + 256 * q + off : k * 1024 + 256 * q + off + 128]

    def x_rhs(b, k):
        t0, nb = blocks[b]
        if b == 0:
            return x0_sb[:, k * B0 : k * B0 + nb]
        o = t0 - B0
        return x12_sb[:, k * T_r + o : k * T_r + o + nb]

    with TileContext(nc) as tc:
        with (
            tc.tile_pool(name="win", bufs=1) as win_p,
            tc.tile_pool(name="warm", bufs=1) as warm_p,
            tc.tile_pool(name="silu_p", bufs=3) as silu_p,
            tc.tile_pool(name="h_p", bufs=2) as h_p,
            tc.tile_pool(name="y_p", bufs=3) as y_p,
            tc.tile_pool(name="ps1", bufs=5, space="PSUM") as ps1_p,
            tc.tile_pool(name="ps2", bufs=3, space="PSUM") as ps2_p,
        ):
            # PE warm-up: dummy matmuls on zeroed SBUF while inputs stream
            # in, lifting the HAM clock gate (cold 1.2 GHz -> warm 2.4 GHz)
            # before the real matmul stream. Results land in the ps2 rotation
            # and are overwritten by later start=True accumulation groups.
            # Warm matmuls 24 and 36 double as timers for the release chain.
            wtile = warm_p.tile([128, 256], bf16, name="wtile", tag="wtile")
            nc.vector.memset(wtile[:], 0.0)
            ps_g24 = ps_g36 = None
            for i in range(44):
                ps_w = ps2_p.tile([128, 512], f32, tag="ps2")
                nc.tensor.matmul(
                    out=ps_w[:, :128],
                    lhsT=wtile[:, 128:256],
                    rhs=wtile[:, 0:128],
                    start=True,
                    stop=True,
                )
                if i == 24:
                    ps_g24 = ps_w
                if i == 36:
                    ps_g36 = ps_w

            # Critical input DMAs (x block0 + pair-0 weights, 1.5 MB): issued
            # via SWDGE on the GpSimd queue, which clears the framework
            # preamble ~1.5us before the Sync/Scalar HWDGE rings — first bytes
            # hit the wire earlier. Ordered by need: wp0, then x0 halves.
            x0_sb = win_p.tile([128, KB1 * B0], bf16, name="x0", tag="x0")
            wp_sb = []
            for p in range(2):
                t = win_p.tile([128, KB1 * 256], bf16, name=f"wp{p}", tag=f"wp{p}")
                wp_sb.append(t)
            nc.gpsimd.dma_start(out=wp_sb[0], in_=wp0[:, :])
            nc.gpsimd.dma_start(out=x0_sb[:, : KH * B0], in_=xb0a[:, :])
            nc.gpsimd.dma_start(out=x0_sb[:, KH * B0 :], in_=xb0b[:, :])
            gate_t = win_p.tile([128, 8], f32, name="gate_t", tag="gate_t")
            nc.vector.tensor_copy(gate_t[:, :4], ps_g24[:, :4])
            nc.vector.tensor_copy(gate_t[:, 4:8], ps_g36[:, :4])

            # Staggered release of the bulk DMAs, a few in flight at a time,
            # in consumption order. Each link: a GpSimd copy into the head of
            # the next DMA's dest tile, gated on data a previous DMA wrote;
            # the dma_start then has a write-after-write dependency on that
            # copy. This keeps the SDMA packet round-robin from letting bulk
            # transfers starve urgent ones.
            whd2_sb = win_p.tile([128, KB1 * 512], bf16, name="whd2", tag="whd2")
            x12_sb = win_p.tile([128, KB1 * T_r], bf16, name="x12", tag="x12")
            wg_sb = [
                win_p.tile([128, KB1 * 1024], bf16, name=f"wg{g}", tag=f"wg{g}")
                for g in range(3)
            ]
            wd_sb = win_p.tile([128, KB2 * 1024], bf16, name="wd", tag="wd")

            def release(dst_tile, src_ap):
                nc.gpsimd.tensor_copy(dst_tile[:, :4], src_ap)

            release(wp_sb[1], gate_t[:, :4])        # after warm matmul 24
            nc.sync.dma_start(out=wp_sb[1], in_=wp1[:, :])
            release(whd2_sb, gate_t[:, 4:8])        # after warm matmul 36
            nc.sync.dma_start(out=whd2_sb, in_=whd2[:, :])
            release(wg_sb[0], gate_t[:, 4:8])       # after warm matmul 36
            nc.sync.dma_start(out=wg_sb[0], in_=wgg[0][:, :])
            release(wg_sb[1], wp_sb[1][:, 8:12])    # after wp1 lands
            nc.sync.dma_start(out=wg_sb[1], in_=wgg[1][:, :])
            release(wg_sb[2], whd2_sb[:, 8:12])     # after whd2 lands
            nc.sync.dma_start(out=wg_sb[2], in_=wgg[2][:, :])
            release(wd_sb, wg_sb[0][:, 8:12])       # after wg0 lands
            nc.sync.dma_start(out=wd_sb, in_=wdp[:, :])
            release(x12_sb, wg_sb[0][:, 12:16])     # after wg0 lands
            nc.scalar.dma_start(out=x12_sb, in_=xb12[:, :])

            h_tiles = {}

            def gateup(b):
                t0, nb = blocks[b]
                for p in range(NPAIR):
                    ps_g = ps1_p.tile([128, 512], f32, tag="ps1")
                    for k in range(KB1):
                        nc.tensor.matmul(
                            out=ps_g[:, :nb],
                            lhsT=gu_lhsT(p, k, False),
                            rhs=x_rhs(b, k),
                            start=(k == 0),
                            stop=(k == KB1 - 1),
                        )
                    ps_u = ps1_p.tile([128, 512], f32, tag="ps1")
                    for k in range(KB1):
                        nc.tensor.matmul(
                            out=ps_u[:, :nb],
                            lhsT=gu_lhsT(p, k, True),
                            rhs=x_rhs(b, k),
                            start=(k == 0),
                            stop=(k == KB1 - 1),
                        )
                    st = silu_p.tile([128, 512], bf16, tag="silu")
                    nc.scalar.activation(
                        st[:, :nb], ps_g[:, :nb], mybir.ActivationFunctionType.Silu
                    )
                    ht = h_p.tile([128, 512], bf16, tag=f"h{p}")
                    nc.vector.tensor_mul(out=ht[:, :nb], in0=ps_u[:, :nb], in1=st[:, :nb])
                    h_tiles[(b, p)] = ht

            def down(b, split_last=False):
                t0, nb = blocks[b]
                for hh in range(NH):
                    # Split the final output tile into two column halves so
                    # the first half's copy+DMA overlaps the second half's
                    # matmuls, shortening the kernel tail.
                    halves = [(0, nb)]
                    if split_last and hh == NH - 1:
                        halves = [(0, nb // 2), (nb // 2, nb - nb // 2)]
                    for ci, (c0, cn) in enumerate(halves):
                        ps_y = ps2_p.tile([128, 512], f32, tag="ps2")
                        for k in range(KB2):
                            nc.tensor.matmul(
                                out=ps_y[:, :cn],
                                lhsT=wd_sb[:, k * 1024 + 128 * hh : k * 1024 + 128 * hh + 128],
                                rhs=h_tiles[(b, k)][:, c0 : c0 + cn],
                                start=(k == 0),
                                stop=(k == KB2 - 1),
                            )
                        yt = y_p.tile([128, 512], f32, tag="y")
                        nc.vector.tensor_copy(yt[:, :cn], ps_y[:, :cn])
                        nc.sync.dma_start(
                            out=yT[:, hh * T_pad + t0 + c0 : hh * T_pad + t0 + c0 + cn],
                            in_=yt[:, :cn],
                        )

            gateup(0)
            gateup(1)
            down(0)
            down(1)
            gateup(2)
            down(2, split_last=True)
    nc.compile()
    return nc


def kernel(hidden_states, local_expert_indices, gate_up_proj, down_proj):
    from concourse.bass_utils import run_bass_kernel_spmd

    x = np.asarray(hidden_states, dtype=np.float32)
    idx = np.asarray(local_expert_indices).astype(np.int64)
    wgu_all = np.asarray(gate_up_proj, dtype=np.float32)
    wd_all = np.asarray(down_proj, dtype=np.float32)

    T, H = x.shape
    E, _, F2 = wgu_all.shape
    F = F2 // 2
    assert E == _NC
    KB1 = H // 128

    order = np.argsort(idx, kind="stable")
    counts = np.bincount(idx, minlength=E)
    starts = np.concatenate([[0], np.cumsum(counts)])
    B0 = 512
    T_pad = max(B0 + 8, int(-(-counts.max() // 4) * 4))
    if (T_pad - B0) % 2:
        T_pad += 2
    T_r = T_pad - B0

    key = (T_pad, H, F)
    if key not in _nc_cache:
        _nc_cache[key] = _build(T_pad, H, F)
    nc = _nc_cache[key]

    x_sorted = x[order]
    in_maps = []
    for e in range(E):
        s, c = int(starts[e]), int(counts[e])
        xb = np.zeros((T_pad, H), np.float32)
        xb[:c] = x_sorted[s : s + c]
        xb = xb.astype(_BF16)
        # x chunk-major: [128 p][KB1 k][t]
        xb0 = np.ascontiguousarray(
            xb[:B0].reshape(B0, KB1, 128).transpose(2, 1, 0)
        ).reshape(128, KB1 * B0)
        xb0a = np.ascontiguousarray(xb0[:, : (KB1 // 2) * B0])
        xb0b = np.ascontiguousarray(xb0[:, (KB1 // 2) * B0 :])
        xb12 = np.ascontiguousarray(
            xb[B0:].reshape(T_r, KB1, 128).transpose(2, 1, 0)
        ).reshape(128, KB1 * T_r)
        # gate/up pair-interleaved columns: packed col block 256p = [gate_p | up_p]
        w = wgu_all[e].astype(_BF16)
        wg_ = w[:, :F].reshape(H, F // 128, 128)
        wu_ = w[:, F:].reshape(H, F // 128, 128)
        wp = np.empty((H, F // 128, 2, 128), _BF16)
        wp[:, :, 0] = wg_
        wp[:, :, 1] = wu_
        wp = wp.reshape(H, 2 * F)
        wpr = wp.reshape(KB1, 128, 2 * F).transpose(1, 0, 2)  # [p][k][c]
        wp0 = np.ascontiguousarray(wpr[:, :, 0:256]).reshape(128, KB1 * 256)
        wp1 = np.ascontiguousarray(wpr[:, :, 256:512]).reshape(128, KB1 * 256)
        whd2 = np.ascontiguousarray(wpr[:, :, 512:1024]).reshape(128, KB1 * 512)
        wggs = {
            f"wgg{g}": np.ascontiguousarray(
                wpr[:, :, 1024 + 1024 * g : 2048 + 1024 * g]
            ).reshape(128, KB1 * 1024)
            for g in range(3)
        }
        wdp = np.ascontiguousarray(
            wd_all[e].astype(_BF16).reshape(F // 128, 128, H).transpose(1, 0, 2)
        ).reshape(128, (F // 128) * H)
        in_maps.append(
            {"xb0a": xb0a, "xb0b": xb0b, "xb12": xb12, "wp0": wp0, "wp1": wp1,
             "whd2": whd2, "wdp": wdp, **wggs}
        )

    res = run_bass_kernel_spmd(nc, in_maps, core_ids=list(range(_NC)))
    global last_run
    last_run = res

    out = np.zeros((T, H), np.float32)
    for e in range(E):
        s, c = int(starts[e]), int(counts[e])
        if c:
            ye = np.asarray(res.results[e]["yT"]).reshape(128, H // 128, T_pad)
            y_pad = ye.transpose(2, 1, 0).reshape(T_pad, H)
            out[order[s : s + c]] = y_pad[:c]
    return out
